# revision 15
# baseline (speedup 1.0000x reference)
"""MemNet Trainium2 kernel v2 (8 cores, batch x vocab = 2x4 sharding).

Core c: sample pair p=c//4 (samples {2p, 2p+1}), vocab quarter q=c%4
(cols [q*8000, (q+1)*8000), padded to 8192 on device).

Structure per core (B_LOC=2 samples, BT=256), 83.1us cost-model time
(3.27x vs the 272us v1 baseline):
- transformer (2 layers) on the core's 2 samples, bf16 matmuls;
  LN via bn_stats/bn_aggr + fast-inverse-sqrt (bit trick + 1 Newton
  step, int ALU ops; pow/sqrt are not HW-legal/table-cheap) + affine on
  the Pool engine; causal mask added into the scores PSUM via an
  identity-matmul of a -30000 upper-triangular tile (PE, ~53ns, kills
  all DVE mask work); exact tanh-gelu in sqrt(a)*u scale: one PSUM
  evacuation, u^2 on Pool, (B+z)*u and (1+tanh)*u as DVE stt, the
  1/sqrt(a) folds moved into the ACT-Tanh scale input and into W2;
  elementwise work spread across ACT/DVE/Pool (Pool = SBUF-only:
  GPSIMD cannot touch PSUM and supports only TT/TSP/copy/memset);
- degenerate memory recurrence (3 fixed-point iters, software-pipelined
  so iter k+1's h-part matmuls run during iter k; see the rank-1
  reduction argument: M0=0 + tied top-k scores => all 512 slots remain
  identical, scan collapses to per-head rank-1 recurrence);
- logits tail in v-major layout: 64 v-tiles of psum [128 vocab, 256 bt],
  3 K-chunk matmuls each (PE-bound at ~20.5us = the per-core flop
  floor), per-partition bias added for free during ACT/DVE evacuation,
  bf16 output staged and DMA'd in 32 pipelined contiguous transfers;
- 11 large DMAs total (one f32 pack, one bf16 pack, split weight packs,
  wlg prefetched during the prelude), ACT exp-table preloaded and the
  PE pstate warmed with dummy matmuls during the startup DMA stall.
"""

import os
from contextlib import ExitStack

import numpy as np
import ml_dtypes

import concourse.bass as bass
import concourse.tile as tile
import concourse.mybir as mybir
from concourse.bass_utils import run_bass_kernel_spmd

F32 = mybir.dt.float32
BF16 = mybir.dt.bfloat16

VOCAB, D, FF, L, NH = 32000, 256, 1024, 2, 8
HD_ATT = D // NH
SLOTS, MD, MH, HD = 512, 128, 4, 32
B, T = 4, 128
DC = D + MD
NCORES = 8
B_LOC = 2                 # samples per core
BT = B_LOC * T            # 256
VSH = VOCAB // 4          # 8000 real vocab per core
VPAD = 8192               # padded
NVT = VPAD // 128         # 64 v-tiles
K_ITERS = 3

AF = mybir.ActivationFunctionType
ALU = mybir.AluOpType
AX = mybir.AxisListType

# f32pack column layout
FP_X0 = 0          # [0,512) x0
FP_B1C = 512       # [512,528) b1 full (L*8)
FP_SB1C = 528      # [528,544) sqrt(a)*b1 full (L*8)
FP_BWV = 544
FP_BER = 545
FP_BG = 546
FP_BLG = 547       # [547,611) blgv (64)
FP_CZ = 611        # [611,613) czb
FP_MAG = 613       # rsqrt magic bits (int32 0x5f375a86 as f32 bits) x2 cols
FP_SH1 = 615       # int32 1 as f32 bits x2 cols
FP_W = 620
GEL_A = 0.7978845608028654 * 0.044715      # cubic coeff in tanh-gelu
GEL_SA = GEL_A ** 0.5
GEL_B = 0.7978845608028654
# bf16pack column layout
BP_MSK = 0         # [0,128) maskneg
BP_IDB = 128       # [128,256) identity
BP_B2 = 256        # [256,768) b2 on partition 0, col 256+l*256
BP_BLK = 768       # [768,896) head-broadcast mask rows 0..3
BP_W = 896


def build_nc():
    PH = int(os.environ.get("KERNEL_PHASE", "3"))
    nc = bass.Bass()

    dp = nc.declare_dram_parameter
    fpk_d = dp("fpk", [128, FP_W], F32, isOutput=False)
    bpk_d = dp("bpk", [128, BP_W], BF16, isOutput=False)
    wtf_d = dp("wtf", [2, 128, 2 * 2048], BF16, isOutput=False)
    w2g_d = dp("w2g", [128, 4096], BF16, isOutput=False)
    wif_d = dp("wif", [128, 3 * 260], BF16, isOutput=False)
    wlg_d = dp("wlg", [3, 128, VPAD], BF16, isOutput=False)
    out_d = dp("out", [8, 128, 2048], BF16, isOutput=True)

    with tile.TileContext(nc) as tc, ExitStack() as ctx:
        pers = ctx.enter_context(tc.tile_pool(name="pers", bufs=1))
        work = ctx.enter_context(tc.tile_pool(name="work", bufs=2))

        def P(shape, dt, tag):
            return pers.tile(shape, dt, tag=tag, name=tag)

        # ---- DMAs (order matters: compute-critical first) ----
        fpk = P([128, FP_W], F32, "fpk")
        nc.sync.dma_start(fpk[:], fpk_d[:, :])
        bpk = P([128, BP_W], BF16, "bpk")
        nc.sync.dma_start(bpk[:], bpk_d[:, :])
        wtf = [P([128, 4096], BF16, f"wtf{c}") for c in range(2)]
        for c in range(2):          # l0 qk+v+wo first so attention starts early
            nc.sync.dma_start(wtf[c][:, 0:1024], wtf_d[c][:, 0:1024])
        for c in range(2):          # l0 w1
            nc.sync.dma_start(wtf[c][:, 1024:2048], wtf_d[c][:, 1024:2048])
        for c in range(2):          # l1
            nc.sync.dma_start(wtf[c][:, 2048:4096], wtf_d[c][:, 2048:4096])
        w2g = P([128, 4096], BF16, "w2g")
        nc.sync.dma_start(w2g[:], w2g_d[:, :])
        wif = P([128, 780], BF16, "wif")
        nc.sync.dma_start(wif[:], wif_d[:, :])
        wlg = [P([128, VPAD], BF16, f"wlg{c}") for c in range(3)]
        for c in range(3):
            nc.sync.dma_start(wlg[c][:], wlg_d[c])

        x = fpk[:, FP_X0:FP_X0 + 512]           # residual stream [128 t, 2*256]
        idb = bpk[:, BP_IDB:BP_IDB + 128]
        mskneg = bpk[:, BP_MSK:BP_MSK + 128]
        czb0 = fpk[:, FP_CZ:FP_CZ + 1]

        # device-built constants
        ones1 = P([1, 128], BF16, "ones1")
        nc.vector.memset(ones1[:], 1.0)
        # PE pstate warm-up: ~3us of back-to-back dummy matmuls during the
        # startup DMA stall so the systolic array is at peak clock when the
        # first real matmuls arrive
        with tc.tile_pool(name="psWRM", bufs=1, space="PSUM") as psWRM:
            wrm = psWRM.tile([128, 128], F32, tag="wrm", name="wrm", bufs=1)
            for _ in range(28):
                nc.tensor.matmul(wrm[:], ones1[:], ones1[:],
                                 start=True, stop=True)
        blk = bpk[0:4, BP_BLK:BP_BLK + 128]
        # preload the exp-family ACT table during the DMA stall
        warm = P([1, 1], BF16, "warm")
        nc.scalar.activation(warm[:], ones1[0:1, 0:1], AF.Exp, bias=czb0[0:1, 0:1], scale=1.0)

        ct = [P([128, BT], BF16, f"ct{c}") for c in range(3)]

        # ---------------- helpers ----------------
        def ln_norm(tag):
            """standardize x per (t, b) over d; return bf16 [128, 512].

            Per-b chains so b0's affine (ACT) completes while b1's stats
            (DVE) still run; b1's affine goes to Pool.
            """
            st = work.tile([128, 12], F32, tag=f"lnst{tag}", name=f"lnst{tag}")
            agg = work.tile([128, 4], F32, tag=f"lnag{tag}", name=f"lnag{tag}")
            ve = work.tile([128, 2], F32, tag=f"lnve{tag}", name=f"lnve{tag}")
            y0 = work.tile([128, 2], F32, tag=f"lny0{tag}", name=f"lny0{tag}")
            pq = work.tile([128, 2], F32, tag=f"lnpq{tag}", name=f"lnpq{tag}")
            rstd = work.tile([128, 2], F32, tag=f"lnrs{tag}", name=f"lnrs{tag}")
            nmr = work.tile([128, 2], F32, tag=f"lnnm{tag}", name=f"lnnm{tag}")
            hbf = work.tile([128, 512], BF16, tag=f"lnhb{tag}", name=f"lnhb{tag}", bufs=1)
            for b in range(B_LOC):
                nc.vector.bn_stats(st[:, 6 * b:6 * b + 6], x[:, 256 * b:256 * (b + 1)])
                nc.vector.bn_aggr(agg[:, 2 * b:2 * b + 2],
                                  st[:, 6 * b:6 * b + 6].rearrange("p (o s) -> p o s", s=6))
            # rstd = rsqrt(var + eps) via bit-trick seed + one Newton step
            # (pow/sqrt are unavailable: pow isn't HW-legal, sqrt costs an
            # ACT table switch). All on Pool, SBUF-only.
            var2 = agg[:].rearrange("p (b s) -> p b s", s=2)[:, :, 1:2]
            mu2 = agg[:].rearrange("p (b s) -> p b s", s=2)[:, :, 0:1]
            nc.gpsimd.tensor_scalar(ve[:], var2, 1e-5, None, op0=ALU.add)
            I32 = mybir.dt.int32
            nc.vector.tensor_tensor(y0[:].bitcast(I32), ve[:].bitcast(I32),
                                    fpk[:, FP_SH1:FP_SH1 + 2].bitcast(I32),
                                    op=ALU.logical_shift_right)
            nc.gpsimd.tensor_sub(y0[:].bitcast(I32),
                                 fpk[:, FP_MAG:FP_MAG + 2].bitcast(I32),
                                 y0[:].bitcast(I32))
            nc.gpsimd.tensor_mul(pq[:], y0[:], y0[:])
            nc.gpsimd.tensor_mul(pq[:], pq[:], ve[:])
            nc.gpsimd.tensor_scalar(pq[:], pq[:], -0.5, 1.5, op0=ALU.mult, op1=ALU.add)
            nc.gpsimd.tensor_mul(rstd[:], y0[:], pq[:])
            nc.gpsimd.tensor_mul(nmr[:], mu2, rstd[:])       # +mu*rstd
            for b in range(B_LOC):
                # x*rstd - mu*rstd
                nc.gpsimd.tensor_scalar(hbf[:, 256 * b:256 * (b + 1)],
                                        x[:, 256 * b:256 * (b + 1)],
                                        rstd[:, b:b + 1], nmr[:, b:b + 1],
                                        op0=ALU.mult, op1=ALU.subtract)
            return hbf

        def transpose_set(pool, hbf, dest, tag):
            """hbf [128 t, (b,d)=512] -> dest[c] [128 d, (b,t)=256] bf16.
            b-outer so b0 transposes start before b1's affine lands."""
            pt = [pool.tile([128, BT], BF16, tag=f"tp{tag}{c}", name=f"tp{tag}{c}", bufs=1)
                  for c in range(2)]
            for b in range(B_LOC):
                for c in range(2):
                    nc.tensor.transpose(pt[c][:, b * 128:(b + 1) * 128],
                                        hbf[:, b * 256 + c * 128: b * 256 + (c + 1) * 128], idb)
            nc.scalar.activation(dest[0][:], pt[0][:], AF.Copy)
            nc.vector.tensor_copy(dest[1][:], pt[1][:])

        # ---------------- transformer ----------------
        for l in range(L):
            lb = l * 2048
            hbf = ln_norm(f"a{l}")
            ht = [work.tile([128, BT], BF16, tag=f"ht{c}", name=f"ht{c}", bufs=1) for c in range(2)]
            q_bf = [work.tile([128, BT], BF16, tag=f"qb{g}", name=f"qb{g}", bufs=1) for g in range(2)]
            k_bf = [work.tile([128, BT], BF16, tag=f"kb{g}", name=f"kb{g}", bufs=1) for g in range(2)]
            v_bf = [work.tile([128, 256], BF16, tag=f"vb{b}", name=f"vb{b}", bufs=1) for b in range(B_LOC)]
            with tc.tile_pool(name=f"psA{l}", bufs=1, space="PSUM") as psA:
                transpose_set(psA, hbf, ht, f"a{l}")
                for m in (0, 2, 1, 3):
                    qkp = psA.tile([128, BT], F32, tag="qkp", name="qkp", bufs=3)
                    for c in range(2):
                        nc.tensor.matmul(qkp[:], wtf[c][:, lb + m * 128:lb + (m + 1) * 128],
                                         ht[c][:], start=(c == 0), stop=(c == 1))
                    dst = (q_bf if m < 2 else k_bf)[m % 2]
                    if m < 2:
                        nc.scalar.activation(dst[:], qkp[:], AF.Copy)
                    else:
                        nc.vector.tensor_copy(dst[:], qkp[:])
                for b in range(B_LOC):
                    vp = psA.tile([128, 256], F32, tag="vp", name="vp", bufs=2)
                    for c in range(2):
                        nc.tensor.matmul(vp[:], ht[c][:, b * 128:(b + 1) * 128],
                                         wtf[c][:, lb + 512:lb + 768], start=(c == 0), stop=(c == 1))
                    if b == 0:
                        nc.vector.tensor_copy(v_bf[b][:], vp[:])
                    else:
                        nc.scalar.activation(v_bf[b][:], vp[:], AF.Copy)

            # attention
            ot = [work.tile([128, BT], BF16, tag=f"ot{g}", name=f"ot{g}", bufs=1) for g in range(2)]
            with tc.tile_pool(name=f"psB{l}", bufs=1, space="PSUM") as psB:
                otp = [psB.tile([128, BT], F32, tag=f"otp{g}", name=f"otp{g}", bufs=1)
                       for g in range(2)]
                # stage-major issue: each engine's program order matches
                # data-readiness (ACT/SP sequencers have no exec queue)
                CH = [(b, g) for b in range(B_LOC) for g in range(2)]
                scp, att, rs, rr, attn = {}, {}, {}, {}, {}
                for ci, (b, g) in enumerate(CH):
                    cols = slice(b * 128, (b + 1) * 128)
                    scp[ci] = psB.tile([128, 512], F32, tag=f"scp{ci}", name=f"scp{ci}", bufs=1)
                    for j in range(4):
                        nc.tensor.matmul(scp[ci][:, j * 128:(j + 1) * 128],
                                         q_bf[g][32 * j:32 * j + 32, cols],
                                         k_bf[g][32 * j:32 * j + 32, cols],
                                         start=True, stop=False,
                                         tile_position=(32 * j, 0))
                        nc.tensor.matmul(scp[ci][:, j * 128:(j + 1) * 128],
                                         idb, mskneg, start=False, stop=True)
                for ci in range(4):
                    att[ci] = work.tile([128, 512], BF16, tag=f"att{ci}", name=f"att{ci}", bufs=1)
                    nc.scalar.activation(att[ci][:], scp[ci][:], AF.Exp, bias=czb0, scale=1.0)
                for ci in range(4):
                    rs[ci] = work.tile([128, 4], F32, tag=f"rs{ci}", name=f"rs{ci}", bufs=1)
                    rr[ci] = work.tile([128, 4], F32, tag=f"rr{ci}", name=f"rr{ci}", bufs=1)
                    for h in range(2):
                        hs = slice(h * 2, h * 2 + 2)
                        nc.vector.tensor_reduce(
                            rs[ci][:, hs],
                            att[ci][:, h * 256:(h + 1) * 256].rearrange("p (j t) -> p j t", t=128),
                            axis=AX.X, op=ALU.add)
                        nc.vector.reciprocal(rr[ci][:, hs], rs[ci][:, hs])
                for ci in range(4):
                    attn[ci] = work.tile([128, 512], BF16, tag=f"attn{ci}", name=f"attn{ci}", bufs=1)
                    for j in range(4):
                        js = slice(j * 128, (j + 1) * 128)
                        nc.gpsimd.tensor_scalar_mul(attn[ci][:, js], att[ci][:, js],
                                                    rr[ci][:, j:j + 1])
                for ci, (b, g) in enumerate(CH):
                    cols = slice(b * 128, (b + 1) * 128)
                    atp = psB.tile([128, 512], BF16, tag="atp", name="atp", bufs=2)
                    for j in range(4):
                        nc.tensor.transpose(atp[:, j * 128:(j + 1) * 128],
                                            attn[ci][:, j * 128:(j + 1) * 128], idb)
                    attb = work.tile([128, 512], BF16, tag="attb", name="attb", bufs=2)
                    if ci % 2 == 0:
                        nc.scalar.activation(attb[:], atp[:], AF.Copy)
                    else:
                        nc.vector.tensor_copy(attb[:], atp[:])
                    for j in range(4):
                        nc.tensor.matmul(otp[g][32 * j:32 * j + 32, cols],
                                         v_bf[b][:, (4 * g + j) * 32:(4 * g + j + 1) * 32],
                                         attb[:, j * 128:(j + 1) * 128],
                                         start=True, stop=True,
                                         tile_position=(0, 32 * j))
                for g in range(2):
                    for b in range(B_LOC):
                        cs = slice(b * 128, (b + 1) * 128)
                        if (2 * g + b) % 2 == 0:
                            nc.scalar.activation(ot[g][:, cs], otp[g][:, cs], AF.Copy)
                        else:
                            nc.vector.tensor_copy(ot[g][:, cs], otp[g][:, cs])
            with tc.tile_pool(name=f"psW{l}", bufs=1, space="PSUM") as psW:
                for b in range(B_LOC):
                    yp = psW.tile([128, 256], F32, tag="yp", name="yp", bufs=2)
                    for g in range(2):
                        nc.tensor.matmul(yp[:], ot[g][:, b * 128:(b + 1) * 128],
                                         wtf[g][:, lb + 768:lb + 1024], start=(g == 0), stop=(g == 1))
                    nc.vector.tensor_add(x[:, b * 256:(b + 1) * 256],
                                         x[:, b * 256:(b + 1) * 256], yp[:])

            # FF
            hbf2 = ln_norm(f"f{l}")
            ht2 = [work.tile([128, BT], BF16, tag=f"h2t{c}", name=f"h2t{c}", bufs=1) for c in range(2)]
            ut = [work.tile([128, BT], BF16, tag=f"ut{ci}", name=f"ut{ci}", bufs=1) for ci in range(8)]
            with tc.tile_pool(name=f"psC{l}", bufs=1, space="PSUM") as psC:
                transpose_set(psC, hbf2, ht2, f"f{l}")
                y2 = [psC.tile([128, 256], F32, tag=f"y2{b}", name=f"y2{b}", bufs=1)
                      for b in range(B_LOC)]
                uus, zzs = {}, {}

                def ff_front(ft):
                    up = psC.tile([128, BT], F32, tag="up", name="up", bufs=4)
                    for c in range(2):
                        nc.tensor.matmul(up[:], wtf[c][:, lb + 1024 + ft * 128:lb + 1024 + (ft + 1) * 128],
                                         ht2[c][:], start=(c == 0), stop=(c == 1))
                    # exact tanh-gelu: psum up = sqrt(a)*(h@W1'); u = up/sqrt(a)+b1
                    # z = (up + sqrt(a)*b1)^2 = a*u^2 ; g = (B + z)*u ; ut = (1+tanh(g))*u
                    # (0.5 folded into w2g)
                    b1a = fpk[:, FP_B1C + l * 8 + ft:FP_B1C + l * 8 + ft + 1]
                    sb1a = fpk[:, FP_SB1C + l * 8 + ft:FP_SB1C + l * 8 + ft + 1]
                    # single psum evacuation: s = up + sqrt(a)*b1 = sqrt(a)*u
                    uu = work.tile([128, BT], F32, tag="gl_u", name="gl_u", bufs=4)
                    if ft % 2 == 0:
                        nc.vector.tensor_scalar_add(uu[:], up[:], sb1a)
                    else:
                        nc.scalar.activation(uu[:], up[:], AF.Identity,
                                             bias=sb1a, scale=1.0)
                    zz = work.tile([128, BT], F32, tag="gl_z", name="gl_z", bufs=4)
                    nc.gpsimd.tensor_mul(zz[:], uu[:], uu[:])
                    uus[ft], zzs[ft] = uu, zz

                def ff_back(ft):
                    uu, zz = uus[ft], zzs[ft]
                    gg = work.tile([128, BT], F32, tag="gl_g", name="gl_g", bufs=4)
                    nc.vector.scalar_tensor_tensor(gg[:], zz[:], GEL_B, uu[:],
                                                   op0=ALU.add, op1=ALU.mult)
                    th = work.tile([128, BT], F32, tag="gl_t", name="gl_t", bufs=4)
                    nc.scalar.activation(th[:], gg[:], AF.Tanh, bias=czb0,
                                         scale=1.0 / GEL_SA)
                    nc.vector.scalar_tensor_tensor(ut[ft][:], th[:], 1.0, uu[:],
                                                   op0=ALU.add, op1=ALU.mult)
                    # consume ut[ft] immediately: w2 partial accumulation
                    for b in range(B_LOC):
                        nc.tensor.matmul(y2[b][:], ut[ft][:, b * 128:(b + 1) * 128],
                                         w2g[:, (ft * 2 + l) * 256:(ft * 2 + l + 1) * 256],
                                         start=(ft == 0), stop=False)

                ff_front(0)
                for ft in range(1, 8):
                    ff_front(ft)
                    ff_back(ft - 1)
                ff_back(7)
                for b in range(B_LOC):
                    nc.tensor.matmul(y2[b][:], ones1[:],
                                     bpk[0:1, BP_B2 + l * 256:BP_B2 + (l + 1) * 256],
                                     start=False, stop=True)
                    nc.vector.tensor_add(x[:, b * 256:(b + 1) * 256],
                                         x[:, b * 256:(b + 1) * 256], y2[b][:])

        # final LN -> ct[0], ct[1]
        hbf = ln_norm("fin")
        with tc.tile_pool(name="psF", bufs=1, space="PSUM") as psF:
            transpose_set(psF, hbf, ct, "fin")

        # ---------------- recurrence (fixed point) ----------------
        nc.vector.memset(ct[2][:], 0.0)
        for b in range(B_LOC):
            nc.gpsimd.memset(ct[2][:, b * 128:b * 128 + 2], 0.0)
        K_IT = K_ITERS if PH >= 2 else 0
        # pre-open the first tail tiles' h-part matmuls (only need ct[0,1]);
        # they overlap the recurrence and are closed by the c2 matmul below
        NPRE = 0
        psLpre = ctx.enter_context(tc.tile_pool(name="psLpre", bufs=1, space="PSUM"))
        pre_lg = []
        for vt in range(NPRE):
            lg = psLpre.tile([128, BT], F32, tag=f"plg{vt}", name=f"plg{vt}", bufs=1)
            for c in range(2):
                nc.tensor.matmul(lg[:], wlg[c][:, vt * 128:(vt + 1) * 128],
                                 ct[c][:], start=(c == 0), stop=False)
            pre_lg.append(lg)
        with tc.tile_pool(name="psR", bufs=1, space="PSUM") as psR:
            # software-pipelined: iter k+1's h-part (c0,c1) matmuls are issued
            # during iter k (they only need ct[0,1]); the c2 matmul closes the
            # accumulation once ct2 from iter k lands. erp chunks go first so
            # the sigmoid chain starts as early as possible.
            tiles = {}

            def rec_open(it):
                wvp = psR.tile([128, BT], F32, tag=f"wvp{it % 2}", name=f"wvp{it % 2}", bufs=1)
                erp = psR.tile([128, BT], F32, tag=f"erp{it % 2}", name=f"erp{it % 2}", bufs=1)
                gp = psR.tile([4, BT], F32, tag=f"gp{it % 2}", name=f"gp{it % 2}", bufs=1)
                last = it == 0      # iter 0 has no c2 part (ct2 starts at zero)
                for c in range(2):
                    st_, sp_ = (c == 0), (c == 1) and last
                    cb = c * 260
                    nc.tensor.matmul(erp[:], wif[:, cb + 128:cb + 256], ct[c][:], start=st_, stop=sp_)
                    nc.tensor.matmul(gp[:], wif[:, cb + 256:cb + 260], ct[c][:], start=st_, stop=sp_)
                    nc.tensor.matmul(wvp[:], wif[:, cb:cb + 128], ct[c][:], start=st_, stop=sp_)
                tiles[it] = (wvp, erp, gp)

            rec_open(0)
            for it in range(K_IT):
                if it + 1 < K_IT:
                    rec_open(it + 1)
                wvp, erp, gp = tiles[it]
                if it > 0:
                    cb = 2 * 260
                    nc.tensor.matmul(erp[:], wif[:, cb + 128:cb + 256], ct[2][:], start=False, stop=True)
                    nc.tensor.matmul(gp[:], wif[:, cb + 256:cb + 260], ct[2][:], start=False, stop=True)
                    nc.tensor.matmul(wvp[:], wif[:, cb:cb + 128], ct[2][:], start=False, stop=True)
                se = work.tile([128, BT], F32, tag="se", name="se")
                nc.scalar.activation(se[:], erp[:], AF.Sigmoid,
                                     bias=fpk[:, FP_BER:FP_BER + 1], scale=1.0)
                s_sb = work.tile([128, BT], F32, tag="ssb", name="ssb")
                nc.gpsimd.tensor_scalar(s_sb[:], se[:], -1.0 / SLOTS, 1.0, op0=ALU.mult, op1=ALU.add)
                wvb = work.tile([128, BT], F32, tag="wvb", name="wvb")
                nc.vector.tensor_scalar(wvb[:], wvp[:], 1.0 / SLOTS,
                                        fpk[:, FP_BWV:FP_BWV + 1], op0=ALU.mult, op1=ALU.add)
                sa = work.tile([4, BT], BF16, tag="sa", name="sa")
                nc.scalar.activation(sa[:], gp[:], AF.Sigmoid,
                                     bias=fpk[0:4, FP_BG:FP_BG + 1], scale=1.0)
                sabc = psR.tile([128, BT], F32, tag="sabc", name="sabc", bufs=1)
                nc.tensor.matmul(sabc[:], blk, sa[:], start=True, stop=True)
                u_sb = work.tile([128, BT], F32, tag="usb", name="usb")
                nc.vector.tensor_mul(u_sb[:], wvb[:], sabc[:])
                msc = work.tile([128, BT], F32, tag="msc", name="msc")
                for b in range(B_LOC):
                    cs = slice(b * 128, (b + 1) * 128)
                    nc.vector.tensor_tensor_scan(msc[:, cs], s_sb[:, cs], u_sb[:, cs],
                                                 0.0, op0=ALU.mult, op1=ALU.add)
                    # prefix-sum scan writes straight into ct2 (bf16, shifted
                    # two steps: logits_t and iface_t use rv = sum_{s<=t-2} m_s)
                    nc.vector.tensor_tensor_scan(ct[2][:, b * 128 + 2:(b + 1) * 128],
                                                 msc[:, b * 128:b * 128 + 126],
                                                 msc[:, b * 128:b * 128 + 126],
                                                 0.0, op0=ALU.add, op1=ALU.bypass)

        # ---------------- logits tail (v-major) ----------------
        if PH < 3:
            nc.sync.dma_start(out_d[0, :, 0:256], ct[0][:])
            nc.sync.dma_start(out_d[1, :, 0:256], ct[2][:])
        stage = [P([128, 512], BF16, f"stg{i}") for i in range(3)]
        with tc.tile_pool(name="psL", bufs=1, space="PSUM") as psL:
            for g in range(32 if PH >= 3 else 0):
                sg = stage[g % 3]
                for j in range(2):
                    vt = g * 2 + j
                    if vt < NPRE:
                        lg = pre_lg[vt]
                        nc.tensor.matmul(lg[:], wlg[2][:, vt * 128:(vt + 1) * 128],
                                         ct[2][:], start=False, stop=True)
                    else:
                        lg = psL.tile([128, BT], F32, tag="lg", name="lg", bufs=8)
                        for c in range(3):
                            nc.tensor.matmul(lg[:], wlg[c][:, vt * 128:(vt + 1) * 128],
                                             ct[c][:], start=(c == 0), stop=(c == 2))
                    dst = sg[:, j * 256:(j + 1) * 256]
                    bia = fpk[:, FP_BLG + vt:FP_BLG + vt + 1]
                    if j % 2 == 0:
                        nc.scalar.activation(dst, lg[:], AF.Identity, bias=bia, scale=1.0)
                    else:
                        nc.vector.tensor_scalar_add(dst, lg[:], bia)
                nc.sync.dma_start(out_d[g // 4, :, (g % 4) * 512:(g % 4) * 512 + 512], sg[:])

    import os as _os2
    _split_excess_waits(nc, maxw=int(_os2.environ.get("MAXW", "1")))
    return nc


def _split_excess_waits(nc, maxw=1):
    """Move overflow sync-waits onto same-engine NoOps (encoding limit)."""
    for fn in nc.m.functions:
        for blk in fn.blocks:
            insts = blk.instructions
            idx = 0
            while idx < len(insts):
                ins = insts[idx]
                si = ins.sync_info
                if si is not None and len(si.on_wait) > maxw:
                    waits = list(si.on_wait)
                    keep = waits[-maxw:]
                    overflow = waits[:-maxw]
                    for jj in range(0, len(overflow), max(maxw, 1)):
                        chunk = overflow[jj:jj + max(maxw, 1)]
                        nop = mybir.InstNoOp(name=nc.get_next_instruction_name(), ins=[], outs=[])
                        nop.engine = ins.engine
                        nop.sync_info = mybir.SyncInfo(on_wait=chunk, on_update=[])
                        nc.register_instruction(nop)
                        insts.insert(idx, nop)
                        idx += 1
                    si.on_wait = keep
                idx += 1


# ---------------- host side ----------------
_NC_CACHE = {}


def _get_nc():
    if "nc" not in _NC_CACHE:
        _NC_CACHE["nc"] = build_nc()
    return _NC_CACHE["nc"]


def prepare_in_maps(input_seq, tok_emb, pos_emb, Wqkv, Wo, ln1_g, ln1_b, ln2_g, ln2_b,
                    W1, b1, W2, b2, lnf_g, lnf_b, W_logits, b_logits, W_iface, b_iface,
                    beta_read, beta_write):
    f = np.float32
    bf = ml_dtypes.bfloat16
    input_seq = np.asarray(input_seq)
    tok_emb = np.asarray(tok_emb, f)
    pos_emb = np.asarray(pos_emb, f)
    Wqkv = np.asarray(Wqkv, f); Wo = np.asarray(Wo, f)
    ln1_g = np.asarray(ln1_g, f)
    ln2_g = np.asarray(ln2_g, f); ln2_b = np.asarray(ln2_b, f)
    W1 = np.asarray(W1, f); b1 = np.asarray(b1, f)
    W2 = np.asarray(W2, f); b2 = np.asarray(b2, f)
    lnf_g = np.asarray(lnf_g, f); lnf_b = np.asarray(lnf_b, f)
    W_logits = np.asarray(W_logits, f); b_logits = np.asarray(b_logits, f)
    W_iface = np.asarray(W_iface, f); b_iface = np.asarray(b_iface, f)

    x0_full = (tok_emb[input_seq] + pos_emb[:T]).astype(f)       # [B, T, D]

    # wtf: per c-chunk [128, l*2048 + (qk 512 | v 256 | wo 256 | w1 1024)]
    wqk = ln1_g[:, :, None] * Wqkv[:, :, :2 * D]
    wqk[:, :, :D] *= f(1.0 / np.sqrt(HD_ATT))
    wvw = ln1_g[:, :, None] * Wqkv[:, :, 2 * D:]
    w1 = f(GEL_SA) * ln2_g[:, :, None] * W1
    wtf = np.zeros((2, 128, 2 * 2048), f)
    for c in range(2):
        rows = slice(c * 128, (c + 1) * 128)
        for l in range(L):
            lb = l * 2048
            wtf[c, :, lb:lb + 512] = wqk[l][rows]
            wtf[c, :, lb + 512:lb + 768] = wvw[l][rows]
            wtf[c, :, lb + 768:lb + 1024] = Wo[l][rows]
            wtf[c, :, lb + 1024:lb + 2048] = w1[l][rows]
    b1c = (b1 + np.einsum("ld,ldf->lf", ln2_b, W1)).reshape(L, 8, 128)  # [l, ft, part]

    w2g = np.zeros((128, 4096), f)
    for ci in range(8):
        for l in range(L):
            w2g[:, (ci * 2 + l) * 256:(ci * 2 + l + 1) * 256] = \
                f(0.5 / GEL_SA) * W2[l][ci * 128:(ci + 1) * 128]

    # W_iface columns
    Wif = W_iface.copy()
    Wif[:D] *= lnf_g[:, None]
    bif_full = b_iface + lnf_b @ W_iface[:D]
    cols_wv, cols_er, cols_g = [], [], []
    for h in range(MH):
        base = h * (4 * HD + 1)
        cols_wv += list(range(base + 2 * HD, base + 3 * HD))
        cols_er += list(range(base + 3 * HD, base + 4 * HD))
        cols_g.append(base + 4 * HD)
    wifp = np.zeros((128, 780), f)
    for c in range(3):
        rows = slice(c * 128, (c + 1) * 128)
        wifp[:, c * 260:c * 260 + 128] = Wif[rows][:, cols_wv]
        wifp[:, c * 260 + 128:c * 260 + 256] = Wif[rows][:, cols_er]
        wifp[:, c * 260 + 256:c * 260 + 260] = Wif[rows][:, cols_g]

    Wlg = W_logits.copy()
    Wlg[:D] *= lnf_g[:, None]
    blg_full = b_logits + lnf_b @ W_logits[:D]

    # bf16 pack
    bpk = np.zeros((128, BP_W), f)
    tri = np.triu(np.ones((128, 128), f), k=1) * f(-30000.0)
    bpk[:, BP_MSK:BP_MSK + 128] = tri
    bpk[:, BP_IDB:BP_IDB + 128] = np.eye(128, dtype=f)
    for l in range(L):
        bpk[0, BP_B2 + l * 256:BP_B2 + (l + 1) * 256] = b2[l]
    for h in range(MH):
        bpk[h, BP_BLK + 32 * h:BP_BLK + 32 * h + 32] = 1.0

    in_maps = []
    for c in range(NCORES):
        p, q = c // 4, c % 4
        fpk = np.zeros((128, FP_W), f)
        x0 = x0_full[2 * p:2 * p + 2]                     # [2, T, D]
        fpk[:, FP_X0:FP_X0 + 512] = x0.transpose(1, 0, 2).reshape(T, 2 * D)
        fpk[:, FP_B1C:FP_B1C + 16] = b1c.reshape(16, 128).T
        fpk[:, FP_SB1C:FP_SB1C + 16] = f(GEL_SA) * b1c.reshape(16, 128).T
        fpk[:, FP_BWV] = bif_full[cols_wv] / SLOTS
        fpk[:, FP_BER] = bif_full[cols_er]
        fpk[0:4, FP_BG] = bif_full[cols_g]
        blgp = np.zeros(VPAD, f)
        blgp[:VSH] = blg_full[q * VSH:(q + 1) * VSH]
        fpk[:, FP_BLG:FP_BLG + 64] = blgp.reshape(64, 128).T
        fpk[:, FP_CZ] = 0.0
        fpk[:, FP_CZ + 1] = 1e-5
        fpk[:, FP_MAG:FP_MAG + 2] = np.int32(0x5F375A86).view(f)
        fpk[:, FP_SH1:FP_SH1 + 2] = np.int32(1).view(f)
        wlgp = np.zeros((3, 128, VPAD), f)
        wsh = Wlg[:, q * VSH:(q + 1) * VSH]
        for ch in range(3):
            wlgp[ch, :, :VSH] = wsh[ch * 128:(ch + 1) * 128]
        in_maps.append({
            "fpk": fpk, "bpk": bpk.astype(bf), "wtf": wtf.astype(bf),
            "w2g": w2g.astype(bf), "wif": wifp.astype(bf),
            "wlg": wlgp.astype(bf),
        })
    return in_maps


def unshard(results):
    out = np.zeros((B, T, VOCAB), np.float32)
    for c in range(NCORES):
        p, q = c // 4, c % 4
        o = np.asarray(results[c]["out"]).reshape(8, 128, 8, 2, 128)
        o = o.transpose(3, 4, 0, 2, 1).reshape(2, 128, VPAD)[:, :, :VSH]
        out[2 * p:2 * p + 2, :, q * VSH:(q + 1) * VSH] = o.astype(np.float32)
    return out


def kernel(**inputs):
    in_maps = prepare_in_maps(**inputs)
    nc = _get_nc()
    res = run_bass_kernel_spmd(nc, in_maps, list(range(NCORES))).results
    return unshard(res)
